# revision 2
# baseline (speedup 1.0000x reference)
"""Distributed Trainium2 kernel for nn_AverageBackProjection (sparse 3-conv chain).

Strategy:
  - Host: reverse-Cuthill-McKee voxel reordering (neighbors land within +-3584
    positions); residual projection folded into W0; voxel dim sharded across 8
    cores with replicated-halo compute (zero collectives); per-tile int16
    ring-slot gather indices precomputed.
  - Device (per core, SPMD, 3 phases): source rows stream through an SBUF
    "token ring" (one 256B bf16 row per token).  Fan-out of the 27 neighbor
    offsets is done by gpsimd dma_gather (SWDGE descriptors, all 16 DMA
    engines) in transpose mode, which yields channel-major [128, 896] bf16
    tiles feeding PSUM-accumulated bf16 matmuls.  Conv outputs are cast to
    bf16, transposed to row-major via the DMA XBAR, and written to DRAM to
    feed the next phase's ring.  The downsampled mean rides on the identity
    (k=13) gather through a tiny selection matmul.
"""

import os
import sys

sys.path.insert(0, "/opt/trn_rl_repo")
os.environ.setdefault("JAX_COMPILATION_CACHE_DIR", "/tmp/jax_cache")
os.environ.setdefault("JAX_PERSISTENT_CACHE_MIN_COMPILE_TIME_SECS", "10")
os.environ.setdefault("JAX_PERSISTENT_CACHE_MIN_ENTRY_SIZE_BYTES", "0")

import numpy as np

N = 300000
NC = 8
NS = N // NC              # 37500 rows per core
C = 128
CO = 32
K = 27
T = 896                   # gather batch (max under the 64-desc SWDGE ring cap)
H = 3584                  # halo reach = 4*T (>= max RCM neighbor distance)
SPAN = 2 * H + T          # 8064 window span per tile
R = 8960                  # ring data tokens = 10*T; zero token at slot R
NE = R + 1
STR = R // 128            # 70 data stripes; zero token in stripe 70
NT2 = 42                  # own-shard tiles  (L2 = 37632 >= 37500)
NT1 = 50                  # conv1 tiles      (L1 = 44800 = L2 + 2H)
NT0 = 58                  # conv0 tiles      (L0 = 51968 = L1 + 2H)
NTW = 66                  # feats window     (LW = 59136 = L0 + 2H)
L2, L1, L0, LW = NT2 * T, NT1 * T, NT0 * T, NTW * T
SKEW = 2 * H // T         # 8: conv0 tile (c+8) holds own-row block c

LAST_EXEC_NS = None
LAST_RESULTS = None


def _bf16(x):
    import ml_dtypes
    return np.asarray(x, dtype=ml_dtypes.bfloat16)


def _perm_rcm(nbr_idx):
    from scipy.sparse import coo_matrix
    from scipy.sparse.csgraph import reverse_cuthill_mckee

    k_idx, n_idx = np.nonzero(nbr_idx < N)
    src = nbr_idx[k_idx, n_idx].astype(np.int64)
    dst = n_idx.astype(np.int64)
    a = coo_matrix((np.ones(len(src), np.int8), (dst, src)), shape=(N, N)).tocsr()
    return np.asarray(reverse_cuthill_mckee(a, symmetric_mode=True), dtype=np.int64)


def _preprocess(feats, W0, W1, W2, nbr_idx):
    feats = np.asarray(feats, np.float32)
    W0 = np.asarray(W0, np.float32)
    W1 = np.asarray(W1, np.float32)
    W2 = np.asarray(W2, np.float32)
    nbr_idx = np.asarray(nbr_idx)

    perm = _perm_rcm(nbr_idx)
    inv = np.empty(N, np.int64)
    inv[perm] = np.arange(N)

    v = nbr_idx[:, perm]
    nbr_new = np.where(v < N, inv[np.clip(v, 0, N - 1)], np.int64(-1))

    d = np.abs(nbr_new - np.arange(N)[None, :])
    maxdist = int(d[nbr_new >= 0].max()) if (nbr_new >= 0).any() else 0
    if maxdist > H:
        raise RuntimeError(f"neighbor distance {maxdist} exceeds halo {H}")

    # fold residual projection A = I - upsample(mean) into W0
    A = np.eye(C, dtype=np.float32) - np.kron(
        np.ones((C // CO, C // CO), np.float32), np.eye(CO, dtype=np.float32)
    ) / (C // CO)
    W0p = np.einsum("ce,kem->kcm", A, W0).astype(np.float32)

    # conv2 weights padded to 128 contraction rows (gathered pad chans = 0)
    W2p = np.zeros((K, C, CO), np.float32)
    W2p[:, :CO, :] = W2

    wd = np.kron(np.ones((C // CO, 1), np.float32),
                 np.eye(CO, dtype=np.float32)) / (C // CO)

    # feats window rows (row-major), bf16, zero-padded outside [0, N)
    feats_p = feats[perm]

    def idx_for(conv_off, nt):
        """Wrapped int16 ring slots [NC, nt, 16, T//16].

        Tile t of this conv covers g = r*NS - conv_off + tT + s; source ns maps
        to ring coord = ns - (r*NS - conv_off - H), slot = coord % R."""
        out = np.empty((NC, nt, 16, K, T // 16), np.int16)
        j = np.arange(nt * T, dtype=np.int64)
        for r in range(NC):
            g = r * NS - conv_off + j
            inb = (g >= 0) & (g < N)
            gc = np.clip(g, 0, N - 1)
            slots = np.empty((K, nt * T), np.int64)
            for k in range(K):
                ns = nbr_new[k, gc]
                val = inb & (ns >= 0)
                coord = ns - (r * NS - conv_off - H)
                slots[k] = np.where(val, coord % R, R)
            out[r] = slots.reshape(K, nt, T // 16, 16).transpose(1, 3, 0, 2)
        return out.astype(np.int16)

    idx0 = idx_for(2 * H, NT0)
    idx1 = idx_for(H, NT1)
    idx2 = idx_for(0, NT2)

    w0b = _bf16(W0p)
    w1b = _bf16(W1)
    w2b = _bf16(W2p)
    wdb = _bf16(wd)

    in_maps = []
    for r in range(NC):
        lo = r * NS - 3 * H
        fw = np.zeros((LW, C), np.float32)
        a, b = max(0, lo), min(N, lo + LW)
        if b > a:
            fw[a - lo:b - lo] = feats_p[a:b]
        in_maps.append({
            "featsw": _bf16(fw),
            "idx0": np.ascontiguousarray(idx0[r]).reshape(NT0, 16, -1),
            "idx1": np.ascontiguousarray(idx1[r]).reshape(NT1, 16, -1),
            "idx2": np.ascontiguousarray(idx2[r]).reshape(NT2, 16, -1),
            "w0": w0b, "w1": w1b, "w2": w2b, "wd": wdb,
        })
    return in_maps, perm


def _build_graph():
    import concourse.bacc as bacc
    import concourse.mybir as mybir
    import concourse.tile as tile

    F32 = mybir.dt.float32
    BF16 = mybir.dt.bfloat16
    I16 = mybir.dt.int16
    NIDX = T // 16  # 56

    nc = bacc.Bacc(None, target_bir_lowering=False, debug=False,
                   num_swdge_queues=4)
    featsw = nc.declare_dram_parameter("featsw", [LW, C], BF16, isOutput=False)
    idx0 = nc.declare_dram_parameter("idx0", [NT0, 16, K * NIDX], I16, isOutput=False)
    idx1 = nc.declare_dram_parameter("idx1", [NT1, 16, K * NIDX], I16, isOutput=False)
    idx2 = nc.declare_dram_parameter("idx2", [NT2, 16, K * NIDX], I16, isOutput=False)
    w0 = nc.declare_dram_parameter("w0", [K, C, C], BF16, isOutput=False)
    w1 = nc.declare_dram_parameter("w1", [K, C, CO], BF16, isOutput=False)
    w2 = nc.declare_dram_parameter("w2", [K, C, CO], BF16, isOutput=False)
    wd = nc.declare_dram_parameter("wd", [C, CO], BF16, isOutput=False)
    out_ext = nc.declare_dram_parameter("out", [CO, L2], F32, isOutput=True)

    out0_dram = nc.dram_tensor("out0_dram", [L0, C], BF16)
    out1_dram = nc.dram_tensor("out1_dram", [L1, C], BF16)
    dT_dram = nc.dram_tensor("dT_dram", [CO, L2], F32)

    def ring_fill(ring, src_dram, row0, nrows, slot0):
        # tokens row0..row0+nrows from row-major DRAM into ring stripes
        s0 = slot0 // 128
        ns = nrows // 128
        nc.sync.dma_start(
            out=ring[:, s0 * 128:(s0 + ns) * 128].rearrange(
                "p (s e) -> p s e", e=128),
            in_=src_dram[row0:row0 + nrows].rearrange(
                "(s p) e -> p s e", p=128),
        )

    with tile.TileContext(nc) as tc:
        with (
            tc.tile_pool(name="pers", bufs=1) as pers,
            tc.tile_pool(name="ip", bufs=4) as ip,
            tc.tile_pool(name="gp", bufs=4) as gp,
            tc.tile_pool(name="st", bufs=3) as st,
            tc.tile_pool(name="oc", bufs=3) as ocp,
            tc.tile_pool(name="ps", bufs=1, space="PSUM") as psp,
            tc.tile_pool(name="psb", bufs=2, space="PSUM") as psbp,
        ):
            wsd = pers.tile([C, CO], BF16, tag="wsd")
            nc.sync.dma_start(out=wsd[:, :], in_=wd[:, :])

            def conv_phase(phase, nt, idx_t, ws, src_dram, cout, dst_dram):
                ring = pers.tile([128, (STR + 1) * 128], BF16, tag="ring")
                nc.vector.memset(ring[:, STR * 128:(STR + 1) * 128], 0.0)
                ring_fill(ring, src_dram, 0, SPAN, 0)
                for t in range(nt):
                    if t > 0:
                        c0 = SPAN + (t - 1) * T
                        ring_fill(ring, src_dram, c0, T, c0 % R)
                    idxs = ip.tile([128, K, NIDX], I16, tag="idx")
                    nc.sync.dma_start(
                        out=idxs[:, :, :].rearrange("p k s -> p (k s)"),
                        in_=idx_t[t].unsqueeze(0).broadcast_to([8, 16, K * NIDX]),
                    )
                    psum = psp.tile([cout, T], F32, tag=f"acc{min(phase,1)}")
                    for k in range(K):
                        g = gp.tile([128, 1, T], BF16, tag="g")
                        nc.gpsimd.dma_gather(
                            out_ap=g[:, :, :], in_ap=ring[:, :],
                            idxs_ap=idxs[:, k, :],
                            num_idxs=T, num_idxs_reg=T, elem_size=C,
                            transpose=True, queue_num=k % 4,
                            sbuf_tokens_per_rank=128,
                            sbuf_free_dim_per_rank=256,
                        )
                        for lo, hi in ((0, 512), (512, T)):
                            nc.tensor.matmul(
                                psum[:, lo:hi], ws[:, k, :], g[:, 0, lo:hi],
                                start=(k == 0), stop=(k == K - 1),
                            )
                        if phase == 0 and k == 13 and SKEW <= t < SKEW + NT2:
                            c = t - SKEW
                            psd = psbp.tile([CO, T], F32, tag="accd")
                            for lo, hi in ((0, 512), (512, T)):
                                nc.tensor.matmul(
                                    psd[:, lo:hi], wsd[:, :], g[:, 0, lo:hi],
                                    start=True, stop=True,
                                )
                            od = ocp.tile([CO, T], F32, tag="od")
                            nc.vector.tensor_copy(od[:, :], psd[:, :])
                            nc.sync.dma_start(
                                out=dT_dram[:, c * T:(c + 1) * T], in_=od[:, :])
                    if phase < 2:
                        # bf16 cast + XBAR transpose to row-major tokens
                        sb = st.tile([128, T], BF16, tag="stg")
                        if cout < 128:
                            nc.vector.memset(sb[:, :], 0.0)
                        nc.vector.tensor_copy(sb[0:cout, :], psum[:, :])
                        tok = st.tile([128, (T // 128) * 128], BF16, tag="tok")
                        for s in range(T // 128):
                            nc.sync.dma_start_transpose(
                                tok[:, s * 128:(s + 1) * 128],
                                sb[:, s * 128:(s + 1) * 128],
                            )
                        nc.sync.dma_start(
                            out=dst_dram[t * T:(t + 1) * T].rearrange(
                                "(s p) e -> p s e", p=128),
                            in_=tok[:, :].rearrange("p (s e) -> p s e", e=128),
                        )
                    else:
                        dtt = ocp.tile([CO, T], F32, tag="dtt")
                        nc.sync.dma_start(
                            out=dtt[:, :], in_=dT_dram[:, t * T:(t + 1) * T])
                        oc = ocp.tile([CO, T], F32, tag="oc")
                        nc.vector.tensor_add(oc[:, :], psum[:, :], dtt[:, :])
                        nc.sync.dma_start(
                            out=out_ext[:, t * T:(t + 1) * T], in_=oc[:, :])

            ws0 = pers.tile([C, K, C], BF16, tag="ws0")
            nc.sync.dma_start(out=ws0[:, :, :], in_=w0.ap().rearrange("k c m -> c k m"))
            conv_phase(0, NT0, idx0, ws0, featsw, C, out0_dram)

            ws1 = pers.tile([C, K, CO], BF16, tag="ws1")
            nc.sync.dma_start(out=ws1[:, :, :], in_=w1.ap().rearrange("k c m -> c k m"))
            conv_phase(1, NT1, idx1, ws1, out0_dram, CO, out1_dram)

            ws2 = pers.tile([C, K, CO], BF16, tag="ws2")
            nc.sync.dma_start(out=ws2[:, :, :], in_=w2.ap().rearrange("k c m -> c k m"))
            conv_phase(2, NT2, idx2, ws2, out1_dram, CO, None)

    nc.compile()
    return nc


def _ensure_ntff_hook():
    """Make `antenv.axon_hooks` importable so bass_utils' trace path works.

    Some images ship an antenv without axon_hooks; the boot degrades
    silently and run_bass_kernel_spmd(trace=True) then dies on import.
    Recreate the module with the same ctypes-backed hook the boot would
    have registered; degrade to a None hook on any failure."""
    import types

    try:
        import antenv.axon_hooks  # noqa: F401
        return
    except Exception:
        pass
    mod = types.ModuleType("antenv.axon_hooks")
    mod._hook = None
    mod.set_axon_ntff_profile_hook = lambda h: setattr(mod, "_hook", h)
    mod.get_axon_ntff_profile_hook = lambda: mod._hook
    try:
        from trn_agent_boot.trn_boot import _ntff_profile_via_ctypes
        mod._hook = _ntff_profile_via_ctypes("/opt/axon/libaxon_pjrt.so")
    except Exception:
        mod._hook = None
    sys.modules["antenv.axon_hooks"] = mod
    try:
        import antenv
        antenv.axon_hooks = mod
    except Exception:
        pass


def kernel(feats, W0, W1, W2, nbr_idx):
    global LAST_EXEC_NS, LAST_RESULTS
    from concourse.bass_utils import run_bass_kernel_spmd

    in_maps, perm = _preprocess(feats, W0, W1, W2, nbr_idx)
    nc = _build_graph()
    trace = os.environ.get("KERNEL_NO_TRACE", "") == ""
    if trace:
        _ensure_ntff_hook()
    try:
        res = run_bass_kernel_spmd(nc, in_maps, core_ids=list(range(NC)), trace=trace)
    except Exception:
        if not trace:
            raise
        res = run_bass_kernel_spmd(nc, in_maps, core_ids=list(range(NC)), trace=False)
    LAST_EXEC_NS = res.exec_time_ns
    LAST_RESULTS = res

    out_p = np.empty((N, CO), np.float32)
    for r in range(NC):
        out_p[r * NS:(r + 1) * NS] = res.results[r]["out"][:, :NS].T
    out = np.empty((N, CO), np.float32)
    out[perm] = out_p
    return out



# revision 9
# speedup vs baseline: 3.1398x; 3.1398x over previous
"""Distributed Trainium2 kernel for nn_AverageBackProjection (sparse 3-conv chain).

Strategy:
  - Host: reverse-Cuthill-McKee voxel reordering (neighbors land within +-3584
    positions); residual projection folded into W0; voxel dim sharded across 8
    cores with replicated-halo compute (zero collectives).
  - Device (per core, SPMD, 3 phases): each conv tile gathers its 27 neighbor
    rows straight from DRAM windows via gpsimd dma_gather in transpose mode
    (channel-major [128, 1024] bf16 tiles feeding PSUM-accumulated matmuls).
    Source arrays carry a 128-row zero block every 8192 data rows; missing
    neighbors index into it, spread across all 128 partitions.  Conv outputs
    are cast to bf16, transposed back to row-major on the tensor engine
    (identity matmul), and written to DRAM for the next phase.  The
    downsampled mean rides on the identity (k=13) gather via a tiny matmul.
"""

import os
import sys

sys.path.insert(0, "/opt/trn_rl_repo")
os.environ.setdefault("JAX_COMPILATION_CACHE_DIR", "/tmp/jax_cache")
os.environ.setdefault("JAX_PERSISTENT_CACHE_MIN_COMPILE_TIME_SECS", "10")
os.environ.setdefault("JAX_PERSISTENT_CACHE_MIN_ENTRY_SIZE_BYTES", "0")

import numpy as np

N = 300000
NC = 8
NS = N // NC              # 37500 rows per core
C = 128
CO = 32
K = 27
T = 896                   # rows per tile (56 descriptors; 64 overflows SWDGE ring)
H = 3584                  # halo reach (max RCM neighbor distance)
NIDX = T // 16            # 56 gather indices per idx partition-row
ZP = 8064                 # data rows between zero blocks (9 tiles)
ZB = 128                  # zero block rows
PP = ZP + ZB              # physical period
W = 12288                 # gather window rows (fits int16, covers span+period)
SKEW = 2 * H // T         # 8: conv0 tile (c+SKEW) holds own-row block c
NT2 = -(-NS // T)                   # 42
NT1 = -(-(NS + 2 * H) // T)         # 50
NT0 = -(-(NS + 4 * H) // T)         # 58
NTW = 66
L2, L1, L0, LW = NT2 * T, NT1 * T, NT0 * T, NTW * T


def _plen(L):
    # physical rows: data + interior zero blocks, ending at the last data row
    return L + ZB * ((L - 1) // ZP)


PL0, PL1, PLW = _plen(L0), _plen(L1), _plen(LW)

LAST_EXEC_NS = None
LAST_RESULTS = None


def _bf16(x):
    import ml_dtypes
    return np.asarray(x, dtype=ml_dtypes.bfloat16)


def _phys(r):
    return r + ZB * (r // ZP)


def _win_base(t, PL):
    return max(0, min(_phys(t * T), PL - W))


def _zrel(t, PL):
    b = _win_base(t, PL)
    j = -(-(b - ZP) // PP)  # smallest j with ZP + PP*j >= b
    zb = ZP + PP * max(0, j)
    assert b <= zb and zb + ZB <= b + W
    return zb - b


def _perm_rcm(nbr_idx):
    from scipy.sparse import coo_matrix
    from scipy.sparse.csgraph import reverse_cuthill_mckee

    k_idx, n_idx = np.nonzero(nbr_idx < N)
    src = nbr_idx[k_idx, n_idx].astype(np.int64)
    dst = n_idx.astype(np.int64)
    a = coo_matrix((np.ones(len(src), np.int8), (dst, src)), shape=(N, N)).tocsr()
    return np.asarray(reverse_cuthill_mckee(a, symmetric_mode=True), dtype=np.int64)


def _preprocess(feats, W0, W1, W2, nbr_idx):
    feats = np.asarray(feats, np.float32)
    W0 = np.asarray(W0, np.float32)
    W1 = np.asarray(W1, np.float32)
    W2 = np.asarray(W2, np.float32)
    nbr_idx = np.asarray(nbr_idx)

    perm = _perm_rcm(nbr_idx)
    inv = np.empty(N, np.int64)
    inv[perm] = np.arange(N)

    v = nbr_idx[:, perm]
    nbr_new = np.where(v < N, inv[np.clip(v, 0, N - 1)], np.int64(-1))

    d = np.abs(nbr_new - np.arange(N)[None, :])
    maxdist = int(d[nbr_new >= 0].max()) if (nbr_new >= 0).any() else 0
    if maxdist > H:
        raise RuntimeError(f"neighbor distance {maxdist} exceeds halo {H}")

    # fold residual projection A = I - upsample(mean) into W0
    A = np.eye(C, dtype=np.float32) - np.kron(
        np.ones((C // CO, C // CO), np.float32), np.eye(CO, dtype=np.float32)
    ) / (C // CO)
    W0p = np.einsum("ce,kem->kcm", A, W0).astype(np.float32)

    # conv2 weights padded to 128 contraction rows (gathered pad chans = 0)
    W2p = np.zeros((K, C, CO), np.float32)
    W2p[:, :CO, :] = W2

    wd = np.kron(np.ones((C // CO, 1), np.float32),
                 np.eye(CO, dtype=np.float32)) / (C // CO)

    feats_p = feats[perm]

    def idx_for(conv_off, nt, src_PL):
        """Window-relative physical gather indices [NC, nt, 16, K*NIDX] int16.

        Tile t of this conv covers g = r*NS - conv_off + t*T + s; source row m
        maps to array-local mloc = m - (r*NS - conv_off - H), physical
        _phys(mloc), window-relative ps - _win_base(t)."""
        out = np.empty((NC, nt, 16, K, NIDX), np.int16)
        j = np.arange(nt * T, dtype=np.int64)
        tt = j // T
        bases = np.array([_win_base(t, src_PL) for t in range(nt)], np.int64)
        zrels = np.array([_zrel(t, src_PL) for t in range(nt)], np.int64)
        zslot = zrels[tt] + (j % T) % ZB
        for r in range(NC):
            g = r * NS - conv_off + j
            inb = (g >= 0) & (g < N)
            gc = np.clip(g, 0, N - 1)
            src_base = r * NS - conv_off - H
            slots = np.empty((K, nt * T), np.int64)
            for k in range(K):
                m = nbr_new[k, gc]
                val = inb & (m >= 0)
                mloc = m - src_base
                ps = _phys(np.clip(mloc, 0, None)) - bases[tt]
                slots[k] = np.where(val, ps, zslot)
            assert slots.min() >= 0 and slots.max() < W
            out[r] = slots.reshape(K, nt, NIDX, 16).transpose(1, 3, 0, 2)
        return out

    idx0 = idx_for(2 * H, NT0, PLW)
    idx1 = idx_for(H, NT1, PL0)
    idx2 = idx_for(0, NT2, PL1)

    w0b = _bf16(W0p)
    w1b = _bf16(W1)
    w2b = _bf16(W2p)
    wdb = _bf16(wd)

    in_maps = []
    for r in range(NC):
        lo = r * NS - 3 * H
        fw = np.zeros((PLW, C), np.float32)
        dloc = np.arange(LW, dtype=np.int64)
        g = lo + dloc
        m = (g >= 0) & (g < N)
        fw[_phys(dloc[m])] = feats_p[g[m]]
        in_maps.append({
            "featsw": _bf16(fw),
            "idx0": np.ascontiguousarray(idx0[r]).reshape(NT0, 16, -1),
            "idx1": np.ascontiguousarray(idx1[r]).reshape(NT1, 16, -1),
            "idx2": np.ascontiguousarray(idx2[r]).reshape(NT2, 16, -1),
            "w0": w0b, "w1": w1b, "w2": w2b, "wd": wdb,
        })
    return in_maps, perm


def _build_graph():
    import concourse.bacc as bacc
    import concourse.mybir as mybir
    import concourse.tile as tile
    from concourse.masks import make_identity

    F32 = mybir.dt.float32
    BF16 = mybir.dt.bfloat16
    I16 = mybir.dt.int16

    nc = bacc.Bacc(None, target_bir_lowering=False, debug=False,
                   num_swdge_queues=4)
    featsw = nc.declare_dram_parameter("featsw", [PLW, C], BF16, isOutput=False)
    idx0 = nc.declare_dram_parameter("idx0", [NT0, 16, K * NIDX], I16, isOutput=False)
    idx1 = nc.declare_dram_parameter("idx1", [NT1, 16, K * NIDX], I16, isOutput=False)
    idx2 = nc.declare_dram_parameter("idx2", [NT2, 16, K * NIDX], I16, isOutput=False)
    w0 = nc.declare_dram_parameter("w0", [K, C, C], BF16, isOutput=False)
    w1 = nc.declare_dram_parameter("w1", [K, C, CO], BF16, isOutput=False)
    w2 = nc.declare_dram_parameter("w2", [K, C, CO], BF16, isOutput=False)
    wd = nc.declare_dram_parameter("wd", [C, CO], BF16, isOutput=False)
    out_ext = nc.declare_dram_parameter("out", [CO, L2], F32, isOutput=True)

    out0_dram = nc.dram_tensor("out0_dram", [PL0, C], BF16)
    out1_dram = nc.dram_tensor("out1_dram", [PL1, C], BF16)
    dT_dram = nc.dram_tensor("dT_dram", [CO, L2], F32)

    with tile.TileContext(nc) as tc:
        with (
            tc.tile_pool(name="pers", bufs=1) as pers,
            tc.tile_pool(name="ip", bufs=4) as ip,
            tc.tile_pool(name="gp", bufs=4) as gp,
            tc.tile_pool(name="st", bufs=3) as st,
            tc.tile_pool(name="oc", bufs=3) as ocp,
            tc.tile_pool(name="ps", bufs=1, space="PSUM") as psp,
            tc.tile_pool(name="psb", bufs=1, space="PSUM") as psbp,
            tc.tile_pool(name="pst", bufs=2, space="PSUM") as pstp,
        ):
            wsd = pers.tile([C, CO], BF16, tag="wsd")
            nc.sync.dma_start(out=wsd[:, :], in_=wd[:, :])
            ident = pers.tile([128, 128], BF16, tag="ident")
            make_identity(nc, ident[:, :])

            # zero the zero blocks of the intermediate arrays
            ztile = pers.tile([128, C], BF16, tag="ztile")
            nc.vector.memset(ztile[:, :], 0.0)
            for dst, PL in ((out0_dram, PL0), (out1_dram, PL1)):
                for zp in range(ZP, PL, PP):
                    nc.sync.dma_start(
                        out=dst[zp:zp + ZB].rearrange("(s p) e -> p s e", p=128),
                        in_=ztile[:, :].unsqueeze(1),
                    )

            qctr = [0]  # global SWDGE issue counter: queue must track the
            # round-robin DMASW sem lane (8 lanes, 4 queues, lane%4==queue)

            def conv_phase(phase, nt, idx_t, ws, src_dram, src_PL, cout,
                           dst_dram):
                for t in range(nt):
                    b = _win_base(t, src_PL)
                    idxs = ip.tile([128, K, NIDX], I16, tag="idx")
                    nc.sync.dma_start(
                        out=idxs[:, :, :].rearrange("p k s -> p (k s)"),
                        in_=idx_t[t].unsqueeze(0).broadcast_to(
                            [8, 16, K * NIDX]),
                    )
                    psum = psp.tile([cout, T], F32, tag=f"acc{min(phase, 1)}")
                    for k in range(K):
                        g = gp.tile([128, 1, T], BF16, tag="g")
                        nc.gpsimd.dma_gather(
                            out_ap=g[:, :, :], in_ap=src_dram[b:b + W, :],
                            idxs_ap=idxs[:, k, :],
                            num_idxs=T, num_idxs_reg=T, elem_size=C,
                            transpose=True, queue_num=qctr[0] % 4,
                        )
                        qctr[0] += 1
                        for lo, hi in ((0, 512), (512, T)):
                            nc.tensor.matmul(
                                psum[:, lo:hi], ws[:, k, :], g[:, 0, lo:hi],
                                start=(k == 0), stop=(k == K - 1),
                            )
                        if phase == 0 and k == 13 and SKEW <= t < SKEW + NT2:
                            c = t - SKEW
                            psd = psbp.tile([CO, T], F32, tag="accd")
                            for lo, hi in ((0, 512), (512, T)):
                                nc.tensor.matmul(
                                    psd[:, lo:hi], wsd[:, :], g[:, 0, lo:hi],
                                    start=True, stop=True,
                                )
                            od = ocp.tile([CO, T], F32, tag="od")
                            nc.vector.tensor_copy(od[:, :], psd[:, :])
                            nc.sync.dma_start(
                                out=dT_dram[:, c * T:(c + 1) * T], in_=od[:, :])
                    if phase < 2:
                        # bf16 cast + PE transpose to row-major tokens
                        sb = st.tile([128, T], BF16, tag="stg")
                        if cout < 128:
                            nc.vector.memset(sb[:, :], 0.0)
                        nc.vector.tensor_copy(sb[0:cout, :], psum[:, :])
                        tok = st.tile([128, T], BF16, tag="tok")
                        for s in range(T // 128):
                            pt = pstp.tile([128, 128], BF16, tag="pt")
                            nc.tensor.transpose(
                                pt[:, :], sb[:, s * 128:(s + 1) * 128],
                                ident[:, :])
                            nc.vector.tensor_copy(
                                tok[:, s * 128:(s + 1) * 128], pt[:, :])
                        pw = _phys(t * T)
                        nc.sync.dma_start(
                            out=dst_dram[pw:pw + T].rearrange(
                                "(s p) e -> p s e", p=128),
                            in_=tok[:, :].rearrange("p (s e) -> p s e", e=128),
                        )
                    else:
                        dtt = ocp.tile([CO, T], F32, tag="dtt")
                        nc.sync.dma_start(
                            out=dtt[:, :], in_=dT_dram[:, t * T:(t + 1) * T])
                        oc = ocp.tile([CO, T], F32, tag="oc")
                        nc.vector.tensor_add(oc[:, :], psum[:, :], dtt[:, :])
                        nc.sync.dma_start(
                            out=out_ext[:, t * T:(t + 1) * T], in_=oc[:, :])

            ws0 = pers.tile([C, K, C], BF16, tag="ws0")
            nc.sync.dma_start(out=ws0[:, :, :], in_=w0.ap().rearrange("k c m -> c k m"))
            conv_phase(0, NT0, idx0, ws0, featsw, PLW, C, out0_dram)

            ws1 = pers.tile([C, K, CO], BF16, tag="ws1")
            nc.sync.dma_start(out=ws1[:, :, :], in_=w1.ap().rearrange("k c m -> c k m"))
            conv_phase(1, NT1, idx1, ws1, out0_dram, PL0, CO, out1_dram)

            ws2 = pers.tile([C, K, CO], BF16, tag="ws2")
            nc.sync.dma_start(out=ws2[:, :, :], in_=w2.ap().rearrange("k c m -> c k m"))
            conv_phase(2, NT2, idx2, ws2, out1_dram, PL1, CO, None)

    nc.compile()
    return nc


def _ensure_ntff_hook():
    """Make `antenv.axon_hooks` importable so bass_utils' trace path works.

    Some images ship an antenv without axon_hooks; the boot degrades
    silently and run_bass_kernel_spmd(trace=True) then dies on import.
    Recreate the module with the same ctypes-backed hook the boot would
    have registered; degrade to a None hook on any failure."""
    import types

    try:
        import antenv.axon_hooks  # noqa: F401
        return
    except Exception:
        pass
    mod = types.ModuleType("antenv.axon_hooks")
    mod._hook = None
    mod.set_axon_ntff_profile_hook = lambda h: setattr(mod, "_hook", h)
    mod.get_axon_ntff_profile_hook = lambda: mod._hook
    try:
        from trn_agent_boot.trn_boot import _ntff_profile_via_ctypes
        mod._hook = _ntff_profile_via_ctypes("/opt/axon/libaxon_pjrt.so")
    except Exception:
        mod._hook = None
    sys.modules["antenv.axon_hooks"] = mod
    try:
        import antenv
        antenv.axon_hooks = mod
    except Exception:
        pass


def kernel(feats, W0, W1, W2, nbr_idx):
    global LAST_EXEC_NS, LAST_RESULTS
    from concourse.bass_utils import run_bass_kernel_spmd

    in_maps, perm = _preprocess(feats, W0, W1, W2, nbr_idx)
    nc = _build_graph()
    trace = os.environ.get("KERNEL_NO_TRACE", "") == ""
    if trace:
        _ensure_ntff_hook()
    try:
        res = run_bass_kernel_spmd(nc, in_maps, core_ids=list(range(NC)), trace=trace)
    except Exception:
        if not trace:
            raise
        res = run_bass_kernel_spmd(nc, in_maps, core_ids=list(range(NC)), trace=False)
    LAST_EXEC_NS = res.exec_time_ns
    LAST_RESULTS = res

    out_p = np.empty((N, CO), np.float32)
    for r in range(NC):
        out_p[r * NS:(r + 1) * NS] = res.results[r]["out"][:, :NS].T
    out = np.empty((N, CO), np.float32)
    out[perm] = out_p
    return out


# revision 14
# speedup vs baseline: 3.2783x; 1.0441x over previous
"""Distributed Trainium2 kernel for nn_AverageBackProjection (sparse 3-conv chain).

Strategy:
  - Host: reverse-Cuthill-McKee voxel reordering (neighbors land within +-3584
    positions); residual projection folded into W0; voxel dim sharded across 8
    cores with replicated-halo compute (zero collectives).
  - Device (per core, SPMD, 3 phases): each conv tile gathers its 27 neighbor
    rows straight from DRAM windows via gpsimd dma_gather in transpose mode
    (channel-major [128, 1024] bf16 tiles feeding PSUM-accumulated matmuls).
    Source arrays carry a 128-row zero block every 8192 data rows; missing
    neighbors index into it, spread across all 128 partitions.  Conv outputs
    are cast to bf16, transposed back to row-major on the tensor engine
    (identity matmul), and written to DRAM for the next phase.  The
    downsampled mean rides on the identity (k=13) gather via a tiny matmul.
"""

import os
import sys

sys.path.insert(0, "/opt/trn_rl_repo")
os.environ.setdefault("JAX_COMPILATION_CACHE_DIR", "/tmp/jax_cache")
os.environ.setdefault("JAX_PERSISTENT_CACHE_MIN_COMPILE_TIME_SECS", "10")
os.environ.setdefault("JAX_PERSISTENT_CACHE_MIN_ENTRY_SIZE_BYTES", "0")

import numpy as np

N = 300000
NC = 8
NS = N // NC              # 37500 rows per core
C = 128
CO = 32
K = 27
T = 896                   # rows per tile (56 descriptors; 64 overflows SWDGE ring)
H = 3584                  # halo reach (max RCM neighbor distance)
NIDX = T // 16            # 56 gather indices per idx partition-row
ZP = 8064                 # data rows between zero blocks (9 tiles)
ZB = 128                  # zero block rows
PP = ZP + ZB              # physical period
W = 12288                 # gather window rows (fits int16, covers span+period)
SKEW = 2 * H // T         # 8: conv0 tile (c+SKEW) holds own-row block c
NT2 = -(-NS // T)                   # 42
NT1 = -(-(NS + 2 * H) // T)         # 50
NT0 = -(-(NS + 4 * H) // T)         # 58
NTW = 66
L2, L1, L0, LW = NT2 * T, NT1 * T, NT0 * T, NTW * T


def _plen(L):
    # physical rows: data + interior zero blocks, ending at the last data row
    return L + ZB * ((L - 1) // ZP)


PL0, PL1, PLW = _plen(L0), _plen(L1), _plen(LW)

LAST_EXEC_NS = None
LAST_RESULTS = None


def _bf16(x):
    import ml_dtypes
    return np.asarray(x, dtype=ml_dtypes.bfloat16)


def _phys(r):
    return r + ZB * (r // ZP)


def _win_base(t, PL):
    return max(0, min(_phys(t * T), PL - W))


def _zrel(t, PL):
    b = _win_base(t, PL)
    j = -(-(b - ZP) // PP)  # smallest j with ZP + PP*j >= b
    zb = ZP + PP * max(0, j)
    assert b <= zb and zb + ZB <= b + W
    return zb - b


def _perm_rcm(nbr_idx):
    from scipy.sparse import coo_matrix
    from scipy.sparse.csgraph import reverse_cuthill_mckee

    k_idx, n_idx = np.nonzero(nbr_idx < N)
    src = nbr_idx[k_idx, n_idx].astype(np.int64)
    dst = n_idx.astype(np.int64)
    a = coo_matrix((np.ones(len(src), np.int8), (dst, src)), shape=(N, N)).tocsr()
    return np.asarray(reverse_cuthill_mckee(a, symmetric_mode=True), dtype=np.int64)


def _preprocess(feats, W0, W1, W2, nbr_idx):
    feats = np.asarray(feats, np.float32)
    W0 = np.asarray(W0, np.float32)
    W1 = np.asarray(W1, np.float32)
    W2 = np.asarray(W2, np.float32)
    nbr_idx = np.asarray(nbr_idx)

    perm = _perm_rcm(nbr_idx)
    inv = np.empty(N, np.int64)
    inv[perm] = np.arange(N)

    v = nbr_idx[:, perm]
    nbr_new = np.where(v < N, inv[np.clip(v, 0, N - 1)], np.int64(-1))

    d = np.abs(nbr_new - np.arange(N)[None, :])
    maxdist = int(d[nbr_new >= 0].max()) if (nbr_new >= 0).any() else 0
    if maxdist > H:
        raise RuntimeError(f"neighbor distance {maxdist} exceeds halo {H}")

    # fold residual projection A = I - upsample(mean) into W0
    A = np.eye(C, dtype=np.float32) - np.kron(
        np.ones((C // CO, C // CO), np.float32), np.eye(CO, dtype=np.float32)
    ) / (C // CO)
    W0p = np.einsum("ce,kem->kcm", A, W0).astype(np.float32)

    # conv2 weights padded to 128 contraction rows (gathered pad chans = 0)
    W2p = np.zeros((K, C, CO), np.float32)
    W2p[:, :CO, :] = W2

    wd = np.kron(np.ones((C // CO, 1), np.float32),
                 np.eye(CO, dtype=np.float32)) / (C // CO)

    feats_p = feats[perm]

    def idx_for(conv_off, nt, src_PL):
        """Window-relative physical gather indices [NC, nt, 16, K*NIDX] int16.

        Tile t of this conv covers g = r*NS - conv_off + t*T + s; source row m
        maps to array-local mloc = m - (r*NS - conv_off - H), physical
        _phys(mloc), window-relative ps - _win_base(t)."""
        out = np.empty((NC, nt, 16, K, NIDX), np.int16)
        j = np.arange(nt * T, dtype=np.int64)
        tt = j // T
        bases = np.array([_win_base(t, src_PL) for t in range(nt)], np.int64)
        zrels = np.array([_zrel(t, src_PL) for t in range(nt)], np.int64)
        zslot = zrels[tt] + (j % T) % ZB
        for r in range(NC):
            g = r * NS - conv_off + j
            inb = (g >= 0) & (g < N)
            gc = np.clip(g, 0, N - 1)
            src_base = r * NS - conv_off - H
            slots = np.empty((K, nt * T), np.int64)
            for k in range(K):
                m = nbr_new[k, gc]
                val = inb & (m >= 0)
                mloc = m - src_base
                ps = _phys(np.clip(mloc, 0, None)) - bases[tt]
                slots[k] = np.where(val, ps, zslot)
            assert slots.min() >= 0 and slots.max() < W
            out[r] = slots.reshape(K, nt, NIDX, 16).transpose(1, 3, 0, 2)
        return out

    idx0 = idx_for(2 * H, NT0, PLW)
    idx1 = idx_for(H, NT1, PL0)
    idx2 = idx_for(0, NT2, PL1)

    w0b = _bf16(W0p)
    w1b = _bf16(W1)
    w2b = _bf16(W2p)
    wdb = _bf16(wd)

    in_maps = []
    for r in range(NC):
        lo = r * NS - 3 * H
        fw = np.zeros((PLW, C), np.float32)
        dloc = np.arange(LW, dtype=np.int64)
        g = lo + dloc
        m = (g >= 0) & (g < N)
        fw[_phys(dloc[m])] = feats_p[g[m]]
        in_maps.append({
            "featsw": _bf16(fw),
            "idx0": np.ascontiguousarray(idx0[r]).reshape(NT0, 16, -1),
            "idx1": np.ascontiguousarray(idx1[r]).reshape(NT1, 16, -1),
            "idx2": np.ascontiguousarray(idx2[r]).reshape(NT2, 16, -1),
            "w0": w0b, "w1": w1b, "w2": w2b, "wd": wdb,
        })
    return in_maps, perm


def _build_graph():
    import concourse.bacc as bacc
    import concourse.mybir as mybir
    import concourse.tile as tile
    from concourse.masks import make_identity

    F32 = mybir.dt.float32
    BF16 = mybir.dt.bfloat16
    I16 = mybir.dt.int16

    nc = bacc.Bacc(None, target_bir_lowering=False, debug=False,
                   num_swdge_queues=4)
    featsw = nc.declare_dram_parameter("featsw", [PLW, C], BF16, isOutput=False)
    idx0 = nc.declare_dram_parameter("idx0", [NT0, 16, K * NIDX], I16, isOutput=False)
    idx1 = nc.declare_dram_parameter("idx1", [NT1, 16, K * NIDX], I16, isOutput=False)
    idx2 = nc.declare_dram_parameter("idx2", [NT2, 16, K * NIDX], I16, isOutput=False)
    w0 = nc.declare_dram_parameter("w0", [K, C, C], BF16, isOutput=False)
    w1 = nc.declare_dram_parameter("w1", [K, C, CO], BF16, isOutput=False)
    w2 = nc.declare_dram_parameter("w2", [K, C, CO], BF16, isOutput=False)
    wd = nc.declare_dram_parameter("wd", [C, CO], BF16, isOutput=False)
    out_ext = nc.declare_dram_parameter("out", [CO, L2], F32, isOutput=True)

    out0_dram = nc.dram_tensor("out0_dram", [PL0, C], BF16)
    out1_dram = nc.dram_tensor("out1_dram", [PL1, C], BF16)
    dT_dram = nc.dram_tensor("dT_dram", [CO, L2], F32)

    with tile.TileContext(nc) as tc:
        with (
            tc.tile_pool(name="pers", bufs=1) as pers,
            tc.tile_pool(name="ip", bufs=4) as ip,
            tc.tile_pool(name="gp", bufs=4) as gp,
            tc.tile_pool(name="st", bufs=3) as st,
            tc.tile_pool(name="oc", bufs=3) as ocp,
            tc.tile_pool(name="ps", bufs=1, space="PSUM") as psp,
            tc.tile_pool(name="psb", bufs=1, space="PSUM") as psbp,
            tc.tile_pool(name="pst", bufs=2, space="PSUM") as pstp,
        ):
            wsd = pers.tile([C, CO], BF16, tag="wsd")
            nc.sync.dma_start(out=wsd[:, :], in_=wd[:, :])
            ident = pers.tile([128, 128], BF16, tag="ident")
            make_identity(nc, ident[:, :])

            # zero the zero blocks of the intermediate arrays
            ztile = pers.tile([128, C], BF16, tag="ztile")
            nc.vector.memset(ztile[:, :], 0.0)
            for dst, PL in ((out0_dram, PL0), (out1_dram, PL1)):
                for zp in range(ZP, PL, PP):
                    nc.sync.dma_start(
                        out=dst[zp:zp + ZB].rearrange("(s p) e -> p s e", p=128),
                        in_=ztile[:, :].unsqueeze(1),
                    )

            qctr = [0]  # global SWDGE issue counter: queue must track the
            # round-robin DMASW sem lane (8 lanes, 4 queues, lane%4==queue)

            def conv_phase(phase, nt, idx_t, ws, src_dram, src_PL, cout,
                           dst_dram):
                for t in range(nt):
                    b = _win_base(t, src_PL)
                    idxs = ip.tile([128, K, NIDX], I16, tag="idx")
                    nc.sync.dma_start(
                        out=idxs[:, :, :].rearrange("p k s -> p (k s)"),
                        in_=idx_t[t].unsqueeze(0).broadcast_to(
                            [8, 16, K * NIDX]),
                    )
                    psum = psp.tile([cout, T], F32, tag=f"acc{min(phase, 1)}")
                    for k in range(K):
                        if k == 13:
                            # identity offset: contiguous rows, no gather.
                            # Row block is tile-aligned (H = 4 tiles) so it
                            # never straddles a zero block.
                            ps0 = _phys(t * T + H)
                            raw = st.tile([128, T // 128, 128], BF16, tag="raw")
                            nc.sync.dma_start(
                                out=raw[:, :, :],
                                in_=src_dram[ps0:ps0 + T].rearrange(
                                    "(s p) e -> p s e", p=128),
                            )
                            gx = gp.tile([128, 1, T], BF16, tag="gx")
                            for s in range(T // 128):
                                pt = pstp.tile([128, 128], BF16, tag="pt")
                                nc.tensor.transpose(
                                    pt[:, :], raw[:, s, :], ident[:, :])
                                nc.vector.tensor_copy(
                                    gx[:, 0, s * 128:(s + 1) * 128], pt[:, :])
                            g = gx
                        else:
                            g = gp.tile([128, 1, T], BF16, tag="g")
                            nc.gpsimd.dma_gather(
                                out_ap=g[:, :, :], in_ap=src_dram[b:b + W, :],
                                idxs_ap=idxs[:, k, :],
                                num_idxs=T, num_idxs_reg=T, elem_size=C,
                                transpose=True, queue_num=qctr[0] % 4,
                            )
                            qctr[0] += 1
                        for lo, hi in ((0, 512), (512, T)):
                            nc.tensor.matmul(
                                psum[:, lo:hi], ws[:, k, :], g[:, 0, lo:hi],
                                start=(k == 0), stop=(k == K - 1),
                            )
                        if phase == 0 and k == 13 and SKEW <= t < SKEW + NT2:
                            c = t - SKEW
                            psd = psbp.tile([CO, T], F32, tag="accd")
                            for lo, hi in ((0, 512), (512, T)):
                                nc.tensor.matmul(
                                    psd[:, lo:hi], wsd[:, :], g[:, 0, lo:hi],
                                    start=True, stop=True,
                                )
                            od = ocp.tile([CO, T], F32, tag="od")
                            nc.vector.tensor_copy(od[:, :], psd[:, :])
                            nc.sync.dma_start(
                                out=dT_dram[:, c * T:(c + 1) * T], in_=od[:, :])
                    if phase < 2:
                        # bf16 cast + PE transpose to row-major tokens
                        sb = st.tile([128, T], BF16, tag="stg")
                        if cout < 128:
                            nc.vector.memset(sb[:, :], 0.0)
                        nc.vector.tensor_copy(sb[0:cout, :], psum[:, :])
                        tok = st.tile([128, T], BF16, tag="tok")
                        for s in range(T // 128):
                            pt = pstp.tile([128, 128], BF16, tag="pt")
                            nc.tensor.transpose(
                                pt[:, :], sb[:, s * 128:(s + 1) * 128],
                                ident[:, :])
                            nc.vector.tensor_copy(
                                tok[:, s * 128:(s + 1) * 128], pt[:, :])
                        pw = _phys(t * T)
                        nc.sync.dma_start(
                            out=dst_dram[pw:pw + T].rearrange(
                                "(s p) e -> p s e", p=128),
                            in_=tok[:, :].rearrange("p (s e) -> p s e", e=128),
                        )
                    else:
                        dtt = ocp.tile([CO, T], F32, tag="dtt")
                        nc.sync.dma_start(
                            out=dtt[:, :], in_=dT_dram[:, t * T:(t + 1) * T])
                        oc = ocp.tile([CO, T], F32, tag="oc")
                        nc.vector.tensor_add(oc[:, :], psum[:, :], dtt[:, :])
                        nc.sync.dma_start(
                            out=out_ext[:, t * T:(t + 1) * T], in_=oc[:, :])

            ws0 = pers.tile([C, K, C], BF16, tag="ws0")
            nc.sync.dma_start(out=ws0[:, :, :], in_=w0.ap().rearrange("k c m -> c k m"))
            conv_phase(0, NT0, idx0, ws0, featsw, PLW, C, out0_dram)

            ws1 = pers.tile([C, K, CO], BF16, tag="ws1")
            nc.sync.dma_start(out=ws1[:, :, :], in_=w1.ap().rearrange("k c m -> c k m"))
            conv_phase(1, NT1, idx1, ws1, out0_dram, PL0, CO, out1_dram)

            ws2 = pers.tile([C, K, CO], BF16, tag="ws2")
            nc.sync.dma_start(out=ws2[:, :, :], in_=w2.ap().rearrange("k c m -> c k m"))
            conv_phase(2, NT2, idx2, ws2, out1_dram, PL1, CO, None)

    nc.compile()
    return nc


def _ensure_ntff_hook():
    """Make `antenv.axon_hooks` importable so bass_utils' trace path works.

    Some images ship an antenv without axon_hooks; the boot degrades
    silently and run_bass_kernel_spmd(trace=True) then dies on import.
    Recreate the module with the same ctypes-backed hook the boot would
    have registered; degrade to a None hook on any failure."""
    import types

    try:
        import antenv.axon_hooks  # noqa: F401
        return
    except Exception:
        pass
    mod = types.ModuleType("antenv.axon_hooks")
    mod._hook = None
    mod.set_axon_ntff_profile_hook = lambda h: setattr(mod, "_hook", h)
    mod.get_axon_ntff_profile_hook = lambda: mod._hook
    try:
        from trn_agent_boot.trn_boot import _ntff_profile_via_ctypes
        mod._hook = _ntff_profile_via_ctypes("/opt/axon/libaxon_pjrt.so")
    except Exception:
        mod._hook = None
    sys.modules["antenv.axon_hooks"] = mod
    try:
        import antenv
        antenv.axon_hooks = mod
    except Exception:
        pass


def kernel(feats, W0, W1, W2, nbr_idx):
    global LAST_EXEC_NS, LAST_RESULTS
    from concourse.bass_utils import run_bass_kernel_spmd

    in_maps, perm = _preprocess(feats, W0, W1, W2, nbr_idx)
    nc = _build_graph()
    trace = os.environ.get("KERNEL_NO_TRACE", "") == ""
    if trace:
        _ensure_ntff_hook()
    try:
        res = run_bass_kernel_spmd(nc, in_maps, core_ids=list(range(NC)), trace=trace)
    except Exception:
        if not trace:
            raise
        res = run_bass_kernel_spmd(nc, in_maps, core_ids=list(range(NC)), trace=False)
    LAST_EXEC_NS = res.exec_time_ns
    LAST_RESULTS = res

    out_p = np.empty((N, CO), np.float32)
    for r in range(NC):
        out_p[r * NS:(r + 1) * NS] = res.results[r]["out"][:, :NS].T
    out = np.empty((N, CO), np.float32)
    out[perm] = out_p
    return out
